# revision 21
# baseline (speedup 1.0000x reference)
"""JSONTreeLSTM Trainium2 kernel: 8-core data-parallel over K=4096 array children.

Layout: transposed — [128 partitions = mem/gate dims, K_loc=512 free = array index].
The number-embedding + running-stat normalization collapses algebraically into the
gate computation: gates = W_hh @ h + u' (x) x_raw_t + v', with
u' = s_c * (W_ih[:,128:] @ w_num), v' = W_ih[:,128:] @ b_num + b_ih + b_hh - m_c*u'
(s_c, m_c = the post-cap running stats, constant for all flat indices >= 100).
The 100 prefix-normalized elements (flat idx < 100 = numbers[0, :100], core 0 only)
are patched into x via x_eff = x_norm/s_c + m_c so the same affine maps them right.

v2 schedule (vs v1 876us baseline):
  - x rows SBUF-resident (one upfront DMA, no per-step DMA)
  - inject matmuls for step t+1 are emitted BEFORE the gate matmuls of step t:
    the PE FIFO is in-order, so putting the state-independent rank-2 inject
    first gives the PE work while it waits on h(t-1) (kills the 2x1.5us/step
    head-of-line stalls seen in the v1 trace)
  - all pointwise tensors bf16 (DVE 2x/4x modes); cell state c bf16
  - tanh(c2) computed directly on ACT (drops the 2*sig(2c)-1 DVE fixup from
    the serial path)
Scan step (128 steps, 2 independent k-chains of KH=256):
  PE:  psum_g(t+1) = u'_g (x) x_{t+1} + v'_g (rank-2, all 4 gates, N=512)
       psum_g(t) += W_hh_g @ h_a(t), W_hh_g @ h_b(t)  (8 matmuls N=256)
  ACT: sg = Sigmoid(psum) over [i,f,o,2g] (N=1024/chain, bf16 out)
  DVE: w=2*sg2-1; m1=sf*c; m2=si*w; c2=m1+m2  (bf16)
  ACT: tc2 = Tanh(c2)
  DVE: h2 = so*tc2 (bf16)
Root: sigmoid(W_fh h + b_fh)*c and h child-sums -> AllReduce -> tree-LSTM root.
"""
import sys

sys.path.insert(0, "/opt/trn_rl_repo")
import numpy as np
import concourse.bacc as bacc
import concourse.mybir as mybir
import concourse.tile as tile
from concourse import bass_utils

F32 = mybir.dt.float32
BF16 = mybir.dt.bfloat16
AF = mybir.ActivationFunctionType
OP = mybir.AluOpType
AX = mybir.AxisListType

K, L, MEM, NCORES = 4096, 128, 128, 8
KLOC = K // NCORES  # 512
STATS_CAP = 100

_compiled = {}


def _build(n_cores=NCORES):
    nc = bacc.Bacc("TRN2", target_bir_lowering=False, debug=False,
                   num_devices=n_cores)

    def din(name, shape):
        return nc.dram_tensor(name, shape, F32, kind="ExternalInput").ap()

    # x rows for all steps: [2, L*KLOC] (row0 = x_eff t-major, row1 = ones)
    xall_d = nc.dram_tensor("xall", [2, L * KLOC], BF16,
                            kind="ExternalInput").ap()
    whhT_d = nc.dram_tensor("whhT", [MEM, 4 * MEM], BF16,
                            kind="ExternalInput").ap()  # W_hh.T, g-block x2
    uvrow_d = nc.dram_tensor("uvrow", [2, 4 * MEM], BF16,
                             kind="ExternalInput").ap()  # rows u', v', g-block x2
    wfhT_d = nc.dram_tensor("wfhT", [MEM, MEM], BF16, kind="ExternalInput").ap()
    bfh_d = din("bfh", [MEM, 1])
    wiouhT_d = din("wiouhT", [MEM, 3 * MEM])
    biouh_d = din("biouh", [MEM, 3])
    wloutT_d = din("wloutT", [MEM, MEM])
    blout_d = din("blout", [MEM, 1])
    out_d = nc.dram_tensor("out", [MEM, 2], F32, kind="ExternalOutput").ap()

    with tile.TileContext(nc) as tc:
        with tc.tile_pool(name="const", bufs=1) as cp, \
             tc.tile_pool(name="state", bufs=4) as sp, \
             tc.tile_pool(name="dram", bufs=1, space="DRAM") as dp:

            xall = cp.tile([2, L * KLOC], BF16, tag="xall")
            whhT = cp.tile([MEM, 4 * MEM], BF16, tag="whhT")
            uvrow = cp.tile([2, 4 * MEM], BF16, tag="uvrow")
            wfhT = cp.tile([MEM, MEM], BF16, tag="wfhT")
            bfh = cp.tile([MEM, 1], F32, tag="bfh")
            wiouhT = cp.tile([MEM, 3 * MEM], F32, tag="wiouhT")
            biouh = cp.tile([MEM, 3], F32, tag="biouh")
            wloutT = cp.tile([MEM, MEM], F32, tag="wloutT")
            blout = cp.tile([MEM, 1], F32, tag="blout")
            for t, d in [(whhT, whhT_d), (uvrow, uvrow_d),
                         (wfhT, wfhT_d), (bfh, bfh_d), (wiouhT, wiouhT_d),
                         (biouh, biouh_d), (wloutT, wloutT_d), (blout, blout_d)]:
                nc.sync.dma_start(t[:], d[:])
            # chunked: a single [2, 65536] transfer overflows the per-AP
            # element-count field and silently drops data
            XCH = 16384
            for q in range(0, L * KLOC, XCH):
                nc.sync.dma_start(xall[:, q:q + XCH], xall_d[:, q:q + XCH])

            # ---- LSTM scan: 2 independent k-chains hide the serial latency ----
            CH = 2
            KH = KLOC // CH
            h = []
            c = []
            for a in range(CH):
                ht = sp.tile([MEM, KH], BF16, tag=f"h{a}", name=f"h{a}_init")
                ct = sp.tile([MEM, KH], BF16, tag=f"c{a}", name=f"c{a}_init")
                nc.any.memset(ht[:], 0.0)
                nc.any.memset(ct[:], 0.0)
                h.append(ht)
                c.append(ct)

            # pre-allocated ping-pong PSUM tiles: the pool-ring otherwise
            # emits a WAR one allocation tighter than the data requires
            # (inject(t+1) was observed waiting on sigma_b(t) instead of
            # sigma_b(t-1)), serializing the PE behind the ACT engine.
            gpfA, _freeA = tc.tile([MEM, 4 * KLOC], F32, space="PSUM",
                                   name="gpfA")
            gpfB, _freeB = tc.tile([MEM, 4 * KLOC], F32, space="PSUM",
                                   name="gpfB")
            gpfP = [gpfA, gpfB]
            # zero stationary row for HAM-warming filler matmuls (accumulate
            # 0 into an open psum group: keeps the PE busy through its
            # dependency stalls so the clock stays at 2.4 GHz, K=8/8)
            zrow = cp.tile([1, MEM], BF16, tag="zrow")
            nc.any.memset(zrow[:], 0.0)

            def filler(t):
                nc.tensor.matmul(gpfP[t % 2][:, 0:KLOC], zrow[:],
                                 xall[:1, 0:KLOC], start=False, stop=False,
                                 skip_group_check=True)

            def inject(t):
                # rank-2 x-injection for step t: all 4 gates, full width.
                # Each matmul covers a full PSUM bank: start=True clears the
                # whole bank, so narrower inject regions are not allowed.
                gpf = gpfP[t % 2]
                xs = xall[:, t * KLOC:(t + 1) * KLOC]
                for j in range(4):
                    nc.tensor.matmul(gpf[:, j * KLOC:(j + 1) * KLOC],
                                     uvrow[:, j * MEM:(j + 1) * MEM],
                                     xs, start=True, stop=False)

            inject(0)
            for t in range(L):
                # manual schedule phases (model-time ordering only): the PE
                # runs inject(t+1) while gates wait on h(t-1); the ACT engine
                # runs sigma_a, tanh_a, sigma_b, tanh_b in that order so
                # chain a's h2 (critical path) is never queued behind
                # sigma_b.
                # PE queue order: gates_a(t) [phase .1] -> inject(t+1)
                # [phase .2] -> gates_b(t) [phase .5]. gates_a is the
                # critical-path op and must never queue behind the inject
                # stream; the inject fills the PE while chain a's
                # sigma/DVE tail runs.
                tc.tile_set_cur_wait(float(t) + 0.2)
                if t + 1 < L:
                    inject(t + 1)
                    tc.tile_set_cur_wait(float(t) + 0.25)
                    filler(t + 1)
                    tc.tile_set_cur_wait(float(t) + 0.55)
                    filler(t + 1)
                    filler(t + 1)
                gpf = gpfP[t % 2]
                gpf3 = gpf[:].rearrange("p (g k) -> p g k", g=4)
                for a in range(CH):
                    ph = float(t) + 0.1 + 0.5 * a
                    ks = slice(a * KH, (a + 1) * KH)
                    tc.tile_set_cur_wait(ph)
                    for j in range(4):
                        nc.tensor.matmul(gpf[:, j * KLOC + a * KH:
                                             j * KLOC + (a + 1) * KH],
                                         whhT[:, j * MEM:(j + 1) * MEM],
                                         h[a][:], start=False, stop=True)
                    sg = sp.tile([MEM, 4 * KH], BF16, tag=f"sg{a}",
                                 name=f"sg{a}_{t}")
                    sg3 = sg[:].rearrange("p (g k) -> p g k", g=4)
                    tc.tile_set_cur_wait(ph + 0.05)
                    nc.scalar.activation(sg3, gpf3[:, :, ks], AF.Sigmoid)
                    si = sg[:, 0:KH]
                    sf = sg[:, KH:2 * KH]
                    sg2 = sg[:, 2 * KH:3 * KH]
                    so = sg[:, 3 * KH:4 * KH]
                    w = sp.tile([MEM, KH], BF16, tag=f"w{a}", name=f"w{a}_{t}")
                    m1 = sp.tile([MEM, KH], BF16, tag=f"m1{a}",
                                 name=f"m1{a}_{t}")
                    c2 = sp.tile([MEM, KH], BF16, tag=f"c{a}", name=f"c{a}_{t}")
                    tc.tile_set_cur_wait(ph + 0.1)
                    nc.vector.tensor_scalar(w, sg2, 2.0, -1.0,
                                            op0=OP.mult, op1=OP.add)
                    nc.vector.tensor_mul(m1, sf, c[a][:])
                    nc.vector.tensor_mul(w, si, w)
                    nc.vector.tensor_add(c2, m1, w)
                    tc2 = sp.tile([MEM, KH], BF16, tag=f"s2c{a}",
                                  name=f"s2c{a}_{t}")
                    # chain b's tanh/h2 are slotted AFTER the next step's
                    # sigma_a on their engines: they only feed gates_b(t+1),
                    # which runs late in the next lap, and ahead of sigma_a
                    # they would stall the critical path ~1us.
                    tc.tile_set_cur_wait(ph + 0.15 if a == 0 else t + 1.18)
                    nc.scalar.activation(tc2, c2[:], AF.Tanh)
                    h2 = sp.tile([MEM, KH], BF16, tag=f"h{a}", name=f"h{a}_{t}")
                    tc.tile_set_cur_wait(ph + 0.2 if a == 0 else t + 1.21)
                    nc.vector.tensor_mul(h2, so, tc2)
                    h[a], c[a] = h2, c2

            # ---- root child-sum ----
            tc.tile_set_cur_wait(float(L) + 1.0)
            part4 = cp.tile([MEM, 4], F32, tag="part4")
            for a in range(CH):
                # reuse scan PSUM (separate banks per chain: start=True
                # clears a full bank, so regions must not share banks)
                fgp = gpfP[0][:, 2 * a * KLOC:2 * a * KLOC + KH]
                nc.tensor.matmul(fgp, wfhT[:], h[a][:], start=True, stop=True)
                fg = sp.tile([MEM, KH], F32, tag=f"sg{a}", name=f"fg{a}")
                nc.scalar.activation(fg, fgp, AF.Sigmoid, bias=bfh[:])
                fc = sp.tile([MEM, KH], F32, tag=f"w{a}", name=f"fc{a}")
                nc.vector.tensor_mul(fc, fg, c[a][:])
                nc.vector.reduce_sum(part4[:, a:a + 1], fc, axis=AX.X)
                nc.vector.reduce_sum(part4[:, 2 + a:3 + a], h[a][:], axis=AX.X)
            part = cp.tile([MEM, 2], F32, tag="part")
            nc.vector.tensor_add(part[:, 0:1], part4[:, 0:1], part4[:, 1:2])
            nc.vector.tensor_add(part[:, 1:2], part4[:, 2:3], part4[:, 3:4])

            bin_ = dp.tile([MEM, 2], F32)
            bout = dp.tile([MEM, 2], F32)
            nc.sync.dma_start(bin_[:], part[:])
            nc.gpsimd.collective_compute(
                "AllReduce", OP.add,
                replica_groups=[list(range(n_cores))],
                ins=[bin_.opt()], outs=[bout.opt()])
            red = cp.tile([MEM, 2], F32, tag="red")
            nc.sync.dma_start(red[:], bout[:])
            fcsum = red[:, 0:1]
            hbar = red[:, 1:2]

            # ---- root tree-LSTM ----
            # iou gates in 3 separate banks of gpfP[1] (start=True bank-clear)
            iou_sl = [gpfP[1][:, j * KLOC:j * KLOC + 1] for j in range(3)]
            for j in range(3):
                nc.tensor.matmul(iou_sl[j], wiouhT[:, j * MEM:(j + 1) * MEM],
                                 hbar, start=True, stop=True)
            rr = cp.tile([MEM, 8], F32, tag="rr")
            i_r = rr[:, 0:1]
            o_r = rr[:, 1:2]
            u_r = rr[:, 2:3]
            nc.scalar.activation(i_r, iou_sl[0], AF.Sigmoid, bias=biouh[:, 0:1])
            nc.scalar.activation(o_r, iou_sl[1], AF.Sigmoid, bias=biouh[:, 1:2])
            nc.scalar.activation(u_r, iou_sl[2], AF.Tanh, bias=biouh[:, 2:3])
            cr = rr[:, 3:4]
            nc.vector.tensor_mul(cr, i_r, u_r)
            nc.vector.tensor_add(cr, cr, fcsum)
            tcr = rr[:, 4:5]
            nc.scalar.activation(tcr, cr, AF.Tanh)
            hr = rr[:, 5:6]
            nc.vector.tensor_mul(hr, o_r, tcr)
            hhp = gpfP[1][:, 3 * KLOC:3 * KLOC + 1]
            nc.tensor.matmul(hhp, wloutT[:], hr, start=True, stop=True)
            outs = cp.tile([MEM, 2], F32, tag="outs")
            nc.vector.tensor_copy(outs[:, 0:1], cr)
            nc.vector.tensor_scalar_add(outs[:, 1:2], hhp, blout[:])
            nc.sync.dma_start(out_d[:], outs[:])
            _freeB()
            _freeA()

    nc.compile()
    return nc


def _prep_inputs(numbers, w_num, b_num, W_ih, W_hh, b_ih, b_hh,
                 W_fh, b_fh, W_iouh, b_iouh, W_lout, b_lout):
    f = np.float32
    numbers = np.ascontiguousarray(numbers, f)

    # Running-stat normalization (reference semantics), first STATS_CAP elems.
    x100 = numbers.reshape(-1)[:STATS_CAP].astype(f)
    kk = np.arange(1, STATS_CAP + 1, dtype=f)
    cs = np.cumsum(x100, dtype=f)
    css = np.cumsum(x100 * x100, dtype=f)
    mean_k = cs / kk
    var_k = np.maximum(css / kk - mean_k * mean_k, 0.0)
    std_k = np.sqrt(var_k)
    use_k = (kk > 3.0) & (std_k > 1e-8)
    inv_k = np.where(use_k, 1.0 / np.where(use_k, std_k, 1.0), 1.0).astype(f)
    x_norm0 = (x100 - mean_k) * inv_k
    m_c = float(mean_k[-1])
    s_c = float(inv_k[-1])

    Wr = np.asarray(W_ih, f)[:, MEM:]                      # [512, 128]
    u = (Wr @ np.asarray(w_num, f)) * s_c
    v = (Wr @ np.asarray(b_num, f) + np.asarray(b_ih, f)
         + np.asarray(b_hh, f) - m_c * u)
    whhT = np.asarray(W_hh, f).T.copy()                    # [128, 512]
    whhT[:, 2 * MEM:3 * MEM] *= 2.0                        # g-gate -> 2g
    u = u.astype(f).copy()
    v = v.astype(f).copy()
    u[2 * MEM:3 * MEM] *= 2.0
    v[2 * MEM:3 * MEM] *= 2.0

    try:
        import ml_dtypes
        bf16 = ml_dtypes.bfloat16
    except ImportError:
        import jax.numpy as jnp
        bf16 = jnp.bfloat16
    shared = {
        "whhT": whhT.astype(bf16),
        "uvrow": np.stack([u, v]).astype(bf16),
        "wfhT": np.asarray(W_fh, f).T.copy().astype(bf16),
        "bfh": np.asarray(b_fh, f).reshape(MEM, 1).copy(),
        "wiouhT": np.asarray(W_iouh, f).T.copy(),
        "biouh": np.asarray(b_iouh, f).reshape(3, MEM).T.copy(),
        "wloutT": np.asarray(W_lout, f).T.copy(),
        "blout": np.asarray(b_lout, f).reshape(MEM, 1).copy(),
    }
    in_maps = []
    for cid in range(NCORES):
        m = dict(shared)
        xT = np.ascontiguousarray(numbers[cid * KLOC:(cid + 1) * KLOC, :].T, f)
        if cid == 0:
            # patch flat elements < 100 (k=0 column -> x row entries [t, 0])
            # so the constant affine reproduces their prefix normalization
            x_eff = x_norm0 / s_c + m_c
            xT[:STATS_CAP, 0] = x_eff
        xaug = np.ones((2, L * KLOC), f)
        xaug[0, :] = xT.reshape(-1)
        m["xall"] = xaug.astype(bf16)
        in_maps.append(m)
    return in_maps


def kernel(**inputs):
    if "nc" not in _compiled:
        _compiled["nc"] = _build()
    nc = _compiled["nc"]
    in_maps = _prep_inputs(**inputs)
    last_err = None
    for _attempt in range(3):
        try:
            res = bass_utils.run_bass_kernel_spmd(nc, in_maps,
                                                  core_ids=list(range(NCORES)))
            break
        except Exception as e:  # transient NRT device faults happen rarely
            last_err = e
    else:
        raise last_err
    out = res.results[0]["out"]                            # [128, 2]
    return np.concatenate([out[:, 0], out[:, 1]])[None, :].astype(np.float32)


# revision 27
# speedup vs baseline: 1.1896x; 1.1896x over previous
"""JSONTreeLSTM Trainium2 kernel: 8-core data-parallel over K=4096 array children.

Layout: transposed — [128 partitions = mem/gate dims, K_loc=512 free = array index].
The number-embedding + running-stat normalization collapses algebraically into the
gate computation: gates = W_hh @ h + u' (x) x_raw_t + v', with
u' = s_c * (W_ih[:,128:] @ w_num), v' = W_ih[:,128:] @ b_num + b_ih + b_hh - m_c*u'
(s_c, m_c = the post-cap running stats, constant for all flat indices >= 100).
The 100 prefix-normalized elements (flat idx < 100 = numbers[0, :100], core 0 only)
are patched into x via x_eff = x_norm/s_c + m_c so the same affine maps them right.

v2 schedule (vs v1 876us baseline):
  - x rows SBUF-resident (one upfront DMA, no per-step DMA)
  - inject matmuls for step t+1 are emitted BEFORE the gate matmuls of step t:
    the PE FIFO is in-order, so putting the state-independent rank-2 inject
    first gives the PE work while it waits on h(t-1) (kills the 2x1.5us/step
    head-of-line stalls seen in the v1 trace)
  - all pointwise tensors bf16 (DVE 2x/4x modes); cell state c bf16
  - tanh(c2) computed directly on ACT (drops the 2*sig(2c)-1 DVE fixup from
    the serial path)
Scan step (128 steps, 2 independent k-chains of KH=256):
  PE:  psum_g(t+1) = u'_g (x) x_{t+1} + v'_g (rank-2, all 4 gates, N=512)
       psum_g(t) += W_hh_g @ h_a(t), W_hh_g @ h_b(t)  (8 matmuls N=256)
  ACT: sg = Sigmoid(psum) over [i,f,o,2g] (N=1024/chain, bf16 out)
  DVE: w=2*sg2-1; m1=sf*c; m2=si*w; c2=m1+m2  (bf16)
  ACT: tc2 = Tanh(c2)
  DVE: h2 = so*tc2 (bf16)
Root: sigmoid(W_fh h + b_fh)*c and h child-sums -> AllReduce -> tree-LSTM root.
"""
import sys

sys.path.insert(0, "/opt/trn_rl_repo")
import numpy as np
import concourse.bacc as bacc
import concourse.mybir as mybir
import concourse.tile as tile
from concourse import bass_utils

F32 = mybir.dt.float32
BF16 = mybir.dt.bfloat16
AF = mybir.ActivationFunctionType
OP = mybir.AluOpType
AX = mybir.AxisListType

K, L, MEM, NCORES = 4096, 128, 128, 8
KLOC = K // NCORES  # 512
STATS_CAP = 100

_compiled = {}


def _build(n_cores=NCORES):
    nc = bacc.Bacc("TRN2", target_bir_lowering=False, debug=False,
                   num_devices=n_cores)

    def din(name, shape):
        return nc.dram_tensor(name, shape, F32, kind="ExternalInput").ap()

    # x rows for all steps: [2, L*KLOC] (row0 = x_eff t-major, row1 = ones)
    xall_d = nc.dram_tensor("xall", [2, L * KLOC], BF16,
                            kind="ExternalInput").ap()
    whhT_d = nc.dram_tensor("whhT", [MEM, 4 * MEM], BF16,
                            kind="ExternalInput").ap()  # W_hh.T, g-block x2
    # uv stationary for row-tiled inject: partitions 32g,32g+1 = u'_g, v'_g
    uvq_d = nc.dram_tensor("uvq", [MEM, MEM], BF16,
                           kind="ExternalInput").ap()
    wfhT_d = nc.dram_tensor("wfhT", [MEM, MEM], BF16, kind="ExternalInput").ap()
    bfh_d = din("bfh", [MEM, 1])
    wiouhT_d = din("wiouhT", [MEM, 3 * MEM])
    biouh_d = din("biouh", [MEM, 3])
    wloutT_d = din("wloutT", [MEM, MEM])
    blout_d = din("blout", [MEM, 1])
    out_d = nc.dram_tensor("out", [MEM, 2], F32, kind="ExternalOutput").ap()

    with tile.TileContext(nc) as tc:
        with tc.tile_pool(name="const", bufs=1) as cp, \
             tc.tile_pool(name="state", bufs=4) as sp, \
             tc.tile_pool(name="dram", bufs=1, space="DRAM") as dp:

            # x replicated at partition pairs {32g, 32g+1} so each row-tiled
            # inject matmul's moving operand aligns with its array row group
            xall = cp.tile([3 * 32 + 2, L * KLOC], BF16, tag="xall")
            whhT = cp.tile([MEM, 4 * MEM], BF16, tag="whhT")
            uvq = cp.tile([MEM, MEM], BF16, tag="uvq")
            wfhT = cp.tile([MEM, MEM], BF16, tag="wfhT")
            bfh = cp.tile([MEM, 1], F32, tag="bfh")
            wiouhT = cp.tile([MEM, 3 * MEM], F32, tag="wiouhT")
            biouh = cp.tile([MEM, 3], F32, tag="biouh")
            wloutT = cp.tile([MEM, MEM], F32, tag="wloutT")
            blout = cp.tile([MEM, 1], F32, tag="blout")
            for t, d in [(whhT, whhT_d), (uvq, uvq_d),
                         (wfhT, wfhT_d), (bfh, bfh_d), (wiouhT, wiouhT_d),
                         (biouh, biouh_d), (wloutT, wloutT_d), (blout, blout_d)]:
                nc.sync.dma_start(t[:], d[:])
            # chunked (a single 65536-element transfer overflows the per-AP
            # element-count field), replicated to each 32-partition group
            XCH = 16384
            for g in range(4):
                for q in range(0, L * KLOC, XCH):
                    nc.sync.dma_start(xall[32 * g:32 * g + 2, q:q + XCH],
                                      xall_d[:, q:q + XCH])

            # ---- LSTM scan: 2 independent k-chains hide the serial latency ----
            CH = 2
            KH = KLOC // CH
            h = []
            c = []
            for a in range(CH):
                ht = sp.tile([MEM, KH], BF16, tag=f"h{a}", name=f"h{a}_init")
                ct = sp.tile([MEM, KH], BF16, tag=f"c{a}", name=f"c{a}_init")
                nc.any.memset(ht[:], 0.0)
                nc.any.memset(ct[:], 0.0)
                h.append(ht)
                c.append(ct)

            # pre-allocated ping-pong PSUM tiles: the pool-ring otherwise
            # emits a WAR one allocation tighter than the data requires
            # (inject(t+1) was observed waiting on sigma_b(t) instead of
            # sigma_b(t-1)), serializing the PE behind the ACT engine.
            gpfA, _freeA = tc.tile([MEM, 4 * KLOC], F32, space="PSUM",
                                   name="gpfA")
            gpfB, _freeB = tc.tile([MEM, 4 * KLOC], F32, space="PSUM",
                                   name="gpfB")
            gpfP = [gpfA, gpfB]

            def inject(t):
                # rank-2 x-injection for step t: all 4 gates, full width.
                # Each matmul covers a full PSUM bank: start=True clears the
                # whole bank, so narrower inject regions are not allowed.
                # tile_position row-tiles the 4 rank-2 matmuls into disjoint
                # 32-row strips of the PE array so they run concurrently
                # (span ~1 matmul instead of 4).
                gpf = gpfP[t % 2]
                for j in range(4):
                    xs = xall[32 * j:32 * j + 2, t * KLOC:(t + 1) * KLOC]
                    nc.tensor.matmul(gpf[:, j * KLOC:(j + 1) * KLOC],
                                     uvq[32 * j:32 * j + 2, :],
                                     xs, start=True, stop=False,
                                     tile_position=(32 * j, 0))

            inject(0)
            for t in range(L):
                # manual schedule phases (model-time ordering only): the PE
                # runs inject(t+1) while gates wait on h(t-1); the ACT engine
                # runs sigma_a, tanh_a, sigma_b, tanh_b in that order so
                # chain a's h2 (critical path) is never queued behind
                # sigma_b.
                # PE queue order: gates_a(t) [phase .1] -> inject(t+1)
                # [phase .2] -> gates_b(t) [phase .5]. gates_a is the
                # critical-path op and must never queue behind the inject
                # stream; the inject fills the PE while chain a's
                # sigma/DVE tail runs.
                tc.tile_set_cur_wait(float(t) + 0.2)
                if t + 1 < L:
                    inject(t + 1)
                gpf = gpfP[t % 2]
                gpf3 = gpf[:].rearrange("p (g k) -> p g k", g=4)
                for a in range(CH):
                    ph = float(t) + 0.1 + 0.5 * a
                    ks = slice(a * KH, (a + 1) * KH)
                    tc.tile_set_cur_wait(ph)
                    for j in range(4):
                        nc.tensor.matmul(gpf[:, j * KLOC + a * KH:
                                             j * KLOC + (a + 1) * KH],
                                         whhT[:, j * MEM:(j + 1) * MEM],
                                         h[a][:], start=False, stop=True)
                    sg = sp.tile([MEM, 4 * KH], BF16, tag=f"sg{a}",
                                 name=f"sg{a}_{t}")
                    sg3 = sg[:].rearrange("p (g k) -> p g k", g=4)
                    tc.tile_set_cur_wait(ph + 0.05)
                    nc.scalar.activation(sg3, gpf3[:, :, ks], AF.Sigmoid)
                    si = sg[:, 0:KH]
                    sf = sg[:, KH:2 * KH]
                    sg2 = sg[:, 2 * KH:3 * KH]
                    so = sg[:, 3 * KH:4 * KH]
                    w = sp.tile([MEM, KH], BF16, tag=f"w{a}", name=f"w{a}_{t}")
                    m1 = sp.tile([MEM, KH], BF16, tag=f"m1{a}",
                                 name=f"m1{a}_{t}")
                    c2 = sp.tile([MEM, KH], BF16, tag=f"c{a}", name=f"c{a}_{t}")
                    tc.tile_set_cur_wait(ph + 0.1)
                    nc.vector.tensor_scalar(w, sg2, 2.0, -1.0,
                                            op0=OP.mult, op1=OP.add)
                    nc.vector.tensor_mul(m1, sf, c[a][:])
                    nc.vector.tensor_mul(w, si, w)
                    nc.vector.tensor_add(c2, m1, w)
                    tc2 = sp.tile([MEM, KH], BF16, tag=f"s2c{a}",
                                  name=f"s2c{a}_{t}")
                    # chain b's tanh/h2 are slotted AFTER the next step's
                    # sigma_a on their engines: they only feed gates_b(t+1),
                    # which runs late in the next lap, and ahead of sigma_a
                    # they would stall the critical path ~1us.
                    tc.tile_set_cur_wait(ph + 0.15 if a == 0 else t + 1.18)
                    nc.scalar.activation(tc2, c2[:], AF.Tanh)
                    h2 = sp.tile([MEM, KH], BF16, tag=f"h{a}", name=f"h{a}_{t}")
                    tc.tile_set_cur_wait(ph + 0.2 if a == 0 else t + 1.21)
                    nc.vector.tensor_mul(h2, so, tc2)
                    h[a], c[a] = h2, c2

            # ---- root child-sum ----
            tc.tile_set_cur_wait(float(L) + 1.0)
            part4 = cp.tile([MEM, 4], F32, tag="part4")
            for a in range(CH):
                # reuse scan PSUM (separate banks per chain: start=True
                # clears a full bank, so regions must not share banks)
                fgp = gpfP[0][:, 2 * a * KLOC:2 * a * KLOC + KH]
                nc.tensor.matmul(fgp, wfhT[:], h[a][:], start=True, stop=True)
                fg = sp.tile([MEM, KH], F32, tag=f"sg{a}", name=f"fg{a}")
                nc.scalar.activation(fg, fgp, AF.Sigmoid, bias=bfh[:])
                fc = sp.tile([MEM, KH], F32, tag=f"w{a}", name=f"fc{a}")
                nc.vector.tensor_mul(fc, fg, c[a][:])
                nc.vector.reduce_sum(part4[:, a:a + 1], fc, axis=AX.X)
                nc.vector.reduce_sum(part4[:, 2 + a:3 + a], h[a][:], axis=AX.X)
            part = cp.tile([MEM, 2], F32, tag="part")
            nc.vector.tensor_add(part[:, 0:1], part4[:, 0:1], part4[:, 1:2])
            nc.vector.tensor_add(part[:, 1:2], part4[:, 2:3], part4[:, 3:4])

            bin_ = dp.tile([MEM, 2], F32)
            bout = dp.tile([MEM, 2], F32)
            nc.sync.dma_start(bin_[:], part[:])
            nc.gpsimd.collective_compute(
                "AllReduce", OP.add,
                replica_groups=[list(range(n_cores))],
                ins=[bin_.opt()], outs=[bout.opt()])
            red = cp.tile([MEM, 2], F32, tag="red")
            nc.sync.dma_start(red[:], bout[:])
            fcsum = red[:, 0:1]
            hbar = red[:, 1:2]

            # ---- root tree-LSTM ----
            # iou gates in 3 separate banks of gpfP[1] (start=True bank-clear)
            iou_sl = [gpfP[1][:, j * KLOC:j * KLOC + 1] for j in range(3)]
            for j in range(3):
                nc.tensor.matmul(iou_sl[j], wiouhT[:, j * MEM:(j + 1) * MEM],
                                 hbar, start=True, stop=True)
            rr = cp.tile([MEM, 8], F32, tag="rr")
            i_r = rr[:, 0:1]
            o_r = rr[:, 1:2]
            u_r = rr[:, 2:3]
            nc.scalar.activation(i_r, iou_sl[0], AF.Sigmoid, bias=biouh[:, 0:1])
            nc.scalar.activation(o_r, iou_sl[1], AF.Sigmoid, bias=biouh[:, 1:2])
            nc.scalar.activation(u_r, iou_sl[2], AF.Tanh, bias=biouh[:, 2:3])
            cr = rr[:, 3:4]
            nc.vector.tensor_mul(cr, i_r, u_r)
            nc.vector.tensor_add(cr, cr, fcsum)
            tcr = rr[:, 4:5]
            nc.scalar.activation(tcr, cr, AF.Tanh)
            hr = rr[:, 5:6]
            nc.vector.tensor_mul(hr, o_r, tcr)
            hhp = gpfP[1][:, 3 * KLOC:3 * KLOC + 1]
            nc.tensor.matmul(hhp, wloutT[:], hr, start=True, stop=True)
            outs = cp.tile([MEM, 2], F32, tag="outs")
            nc.vector.tensor_copy(outs[:, 0:1], cr)
            nc.vector.tensor_scalar_add(outs[:, 1:2], hhp, blout[:])
            nc.sync.dma_start(out_d[:], outs[:])
            _freeB()
            _freeA()

    nc.compile()
    return nc


def _prep_inputs(numbers, w_num, b_num, W_ih, W_hh, b_ih, b_hh,
                 W_fh, b_fh, W_iouh, b_iouh, W_lout, b_lout):
    f = np.float32
    numbers = np.ascontiguousarray(numbers, f)

    # Running-stat normalization (reference semantics), first STATS_CAP elems.
    x100 = numbers.reshape(-1)[:STATS_CAP].astype(f)
    kk = np.arange(1, STATS_CAP + 1, dtype=f)
    cs = np.cumsum(x100, dtype=f)
    css = np.cumsum(x100 * x100, dtype=f)
    mean_k = cs / kk
    var_k = np.maximum(css / kk - mean_k * mean_k, 0.0)
    std_k = np.sqrt(var_k)
    use_k = (kk > 3.0) & (std_k > 1e-8)
    inv_k = np.where(use_k, 1.0 / np.where(use_k, std_k, 1.0), 1.0).astype(f)
    x_norm0 = (x100 - mean_k) * inv_k
    m_c = float(mean_k[-1])
    s_c = float(inv_k[-1])

    Wr = np.asarray(W_ih, f)[:, MEM:]                      # [512, 128]
    u = (Wr @ np.asarray(w_num, f)) * s_c
    v = (Wr @ np.asarray(b_num, f) + np.asarray(b_ih, f)
         + np.asarray(b_hh, f) - m_c * u)
    whhT = np.asarray(W_hh, f).T.copy()                    # [128, 512]
    whhT[:, 2 * MEM:3 * MEM] *= 2.0                        # g-gate -> 2g
    u = u.astype(f).copy()
    v = v.astype(f).copy()
    u[2 * MEM:3 * MEM] *= 2.0
    v[2 * MEM:3 * MEM] *= 2.0

    try:
        import ml_dtypes
        bf16 = ml_dtypes.bfloat16
    except ImportError:
        import jax.numpy as jnp
        bf16 = jnp.bfloat16
    uvq = np.zeros((MEM, MEM), np.float32)
    for g in range(4):
        uvq[32 * g] = u[g * MEM:(g + 1) * MEM]
        uvq[32 * g + 1] = v[g * MEM:(g + 1) * MEM]
    shared = {
        "whhT": whhT.astype(bf16),
        "uvq": uvq.astype(bf16),
        "wfhT": np.asarray(W_fh, f).T.copy().astype(bf16),
        "bfh": np.asarray(b_fh, f).reshape(MEM, 1).copy(),
        "wiouhT": np.asarray(W_iouh, f).T.copy(),
        "biouh": np.asarray(b_iouh, f).reshape(3, MEM).T.copy(),
        "wloutT": np.asarray(W_lout, f).T.copy(),
        "blout": np.asarray(b_lout, f).reshape(MEM, 1).copy(),
    }
    in_maps = []
    for cid in range(NCORES):
        m = dict(shared)
        xT = np.ascontiguousarray(numbers[cid * KLOC:(cid + 1) * KLOC, :].T, f)
        if cid == 0:
            # patch flat elements < 100 (k=0 column -> x row entries [t, 0])
            # so the constant affine reproduces their prefix normalization
            x_eff = x_norm0 / s_c + m_c
            xT[:STATS_CAP, 0] = x_eff
        xaug = np.ones((2, L * KLOC), f)
        xaug[0, :] = xT.reshape(-1)
        m["xall"] = xaug.astype(bf16)
        in_maps.append(m)
    return in_maps


def kernel(**inputs):
    if "nc" not in _compiled:
        _compiled["nc"] = _build()
    nc = _compiled["nc"]
    in_maps = _prep_inputs(**inputs)
    last_err = None
    for _attempt in range(3):
        try:
            res = bass_utils.run_bass_kernel_spmd(nc, in_maps,
                                                  core_ids=list(range(NCORES)))
            break
        except Exception as e:  # transient NRT device faults happen rarely
            last_err = e
    else:
        raise last_err
    out = res.results[0]["out"]                            # [128, 2]
    return np.concatenate([out[:, 0], out[:, 1]])[None, :].astype(np.float32)
